# revision 2
# baseline (speedup 1.0000x reference)
"""Trainium2 Bass kernel for nn_BinaryLinear: out = sign(x @ sign(W).T + bias).

Strategy
--------
Data-parallel over the 8192-token dim: each of the 8 cores gets 1024 tokens
and the full weight matrix.

On-chip compute (per core) is the NT GEMM z.T = sign(W) @ x.T done on the
TensorEngine with the contraction (in_features) on the partition dim:

  psum[outf, tok] = sum_k w_b_T[k, outf] * x_T[k, tok]

Both operands are pre-transposed on the host (pure layout prep) so every DMA
is contiguous-per-partition. x is split into bf16 hi/lo halves
(x ~= hi + lo, ~17 mantissa bits) and both halves are accumulated into the
same fp32 PSUM, giving error at/below the fp32 reference's own accumulation
error while running the PE at full bf16 rate (fp32 matmul is 4x slower).
sign(W) is computed on-chip (ScalarE Sign activation, fp32 -> bf16 +-1.0,
exactly representable). Epilogue fuses bias-add + sign + PSUM->SBUF in one
ScalarE activation op (bias is per-partition in the z.T layout).

Output is written as z.T [out_features, tokens] per core and untransposed on
the host.
"""

import numpy as np
import ml_dtypes

import concourse.tile as tile
import concourse.mybir as mybir
from concourse import bacc
from concourse.bass_utils import run_bass_kernel_spmd

N_CORES = 8
N_TOK = 8192
D_IN = 4096
D_OUT = 4096
P = 128
T = N_TOK // N_CORES  # 1024 tokens per core
KT = D_IN // P  # 32 contraction tiles
MT = D_OUT // P  # 32 out-feature tiles
M2 = 2  # m-tiles per cached W block (256 outf cols -> 1KB DMA lines)
MB = MT // M2  # 16 W blocks
TB = 512  # token block (one PSUM bank of fp32)
NB = T // TB  # 2 token blocks per core

F32 = mybir.dt.float32
BF16 = mybir.dt.bfloat16
SIGN = mybir.ActivationFunctionType.Sign

_nc_cache = None


def build():
    """Build + compile the per-core Bass/Tile module (SPMD: same on all cores)."""
    global _nc_cache
    if _nc_cache is not None:
        return _nc_cache
    nc = bacc.Bacc("TRN2", target_bir_lowering=False, debug=False, num_devices=N_CORES)
    xhi_d = nc.dram_tensor("x_hi_t", [D_IN, T], BF16, kind="ExternalInput").ap()
    xlo_d = nc.dram_tensor("x_lo_t", [D_IN, T], BF16, kind="ExternalInput").ap()
    w_d = nc.dram_tensor("w_t", [D_IN, D_OUT], F32, kind="ExternalInput").ap()
    b_d = nc.dram_tensor("bias", [D_OUT], F32, kind="ExternalInput").ap()
    out_d = nc.dram_tensor("out_t", [D_OUT, T], F32, kind="ExternalOutput").ap()

    with tile.TileContext(nc) as tc:
        with (
            tc.tile_pool(name="x", bufs=1) as x_pool,
            tc.tile_pool(name="wstage", bufs=4) as wstage_pool,
            tc.tile_pool(name="wsb", bufs=2) as w_pool,
            tc.tile_pool(name="bias", bufs=1) as b_pool,
            tc.tile_pool(name="out", bufs=4) as out_pool,
            tc.tile_pool(name="psum", bufs=6, space="PSUM") as psum_pool,
        ):
            # Resident x (both splits): [P, KT, T] bf16, 64KB/partition each.
            xhi = x_pool.tile([P, KT, T], BF16, tag="xhi")
            xlo = x_pool.tile([P, KT, T], BF16, tag="xlo")
            for ko in range(KT):
                nc.sync.dma_start(xhi[:, ko, :], xhi_d[ko * P : (ko + 1) * P, :])
                nc.sync.dma_start(xlo[:, ko, :], xlo_d[ko * P : (ko + 1) * P, :])
            # bias, outf-partition-major: bias_sb[p, mo] = bias[mo*128 + p]
            bias_sb = b_pool.tile([P, MT], F32, tag="bias")
            nc.sync.dma_start(bias_sb[:], b_d.rearrange("(mo p) -> p mo", p=P))

            for mb in range(MB):
                # Stage + sign-convert a [D_IN, 256] W column block to bf16.
                wsb = w_pool.tile([P, KT, M2 * P], BF16, tag="wsb")
                for k in range(KT):
                    wstage = wstage_pool.tile([P, M2 * P], F32, tag="wstage")
                    nc.sync.dma_start(
                        wstage[:],
                        w_d[k * P : (k + 1) * P, mb * M2 * P : (mb + 1) * M2 * P],
                    )
                    nc.scalar.activation(wsb[:, k, :], wstage[:], SIGN)
                for n in range(NB):
                    psums = [
                        psum_pool.tile([P, TB], F32, tag="psum", name=f"psum{mi}")
                        for mi in range(M2)
                    ]
                    for k in range(KT):
                        for mi in range(M2):
                            lhsT = wsb[:, k, mi * P : (mi + 1) * P]
                            nc.tensor.matmul(
                                psums[mi][:],
                                lhsT,
                                xhi[:, k, n * TB : (n + 1) * TB],
                                start=(k == 0),
                                stop=False,
                            )
                            nc.tensor.matmul(
                                psums[mi][:],
                                lhsT,
                                xlo[:, k, n * TB : (n + 1) * TB],
                                start=False,
                                stop=(k == KT - 1),
                            )
                    for mi in range(M2):
                        m = mb * M2 + mi
                        osb = out_pool.tile([P, TB], F32, tag="osb")
                        nc.scalar.activation(
                            osb[:], psums[mi][:], SIGN, bias=bias_sb[:, m : m + 1]
                        )
                        nc.sync.dma_start(
                            out_d[m * P : (m + 1) * P, n * TB : (n + 1) * TB], osb[:]
                        )
    nc.compile()
    _nc_cache = nc
    return nc


def prep_in_maps(x, weight, bias):
    """Host-side layout prep: bf16 hi/lo split of x, transposes, token shards."""
    x = np.asarray(x, dtype=np.float32)
    weight = np.asarray(weight, dtype=np.float32)
    bias = np.asarray(bias, dtype=np.float32)

    x_hi = x.astype(ml_dtypes.bfloat16)
    x_lo = (x - x_hi.astype(np.float32)).astype(ml_dtypes.bfloat16)
    xhi_t = np.ascontiguousarray(x_hi.T)  # [D_IN, N_TOK]
    xlo_t = np.ascontiguousarray(x_lo.T)
    w_t = np.ascontiguousarray(weight.T)  # [D_IN, D_OUT]

    in_maps = []
    for c in range(N_CORES):
        sl = slice(c * T, (c + 1) * T)
        in_maps.append(
            {
                "x_hi_t": np.ascontiguousarray(xhi_t[:, sl]),
                "x_lo_t": np.ascontiguousarray(xlo_t[:, sl]),
                "w_t": w_t,
                "bias": bias,
            }
        )
    return in_maps


def run(x, weight, bias, **spmd_kwargs):
    """Run on the 8 cores; returns (full_output, BassKernelResults)."""
    nc = build()
    in_maps = prep_in_maps(x, weight, bias)
    res = run_bass_kernel_spmd(nc, in_maps, core_ids=list(range(N_CORES)), **spmd_kwargs)
    out = np.empty((N_TOK, D_OUT), dtype=np.float32)
    for c in range(N_CORES):
        out[c * T : (c + 1) * T, :] = res.results[c]["out_t"].T
    return out, res


def kernel(x, weight, bias):
    out, _ = run(x, weight, bias)
    return out


# revision 4
# speedup vs baseline: 1.0193x; 1.0193x over previous
"""Trainium2 Bass kernel for nn_BinaryLinear: out = sign(x @ sign(W).T + bias).

Strategy
--------
Data-parallel over the 8192-token dim: each of the 8 cores gets 1024 tokens
and the full weight matrix.

On-chip compute (per core) is the NT GEMM z.T = sign(W) @ x.T done on the
TensorEngine with the contraction (in_features) on the partition dim:

  psum[outf, tok] = sum_k w_b_T[k, outf] * x_T[k, tok]

Both operands are pre-transposed on the host (pure layout prep) so every DMA
is contiguous-per-partition. x is split into bf16 hi/lo halves
(x ~= hi + lo, ~17 mantissa bits) and both halves are accumulated into the
same fp32 PSUM, giving error at/below the fp32 reference's own accumulation
error while running the PE at full bf16 rate (fp32 matmul is 4x slower).
sign(W) is computed on-chip (ScalarE Sign activation, fp32 -> bf16 +-1.0,
exactly representable). Epilogue fuses bias-add + sign + PSUM->SBUF in one
ScalarE activation op (bias is per-partition in the z.T layout).

Output is written as z.T [out_features, tokens] per core and untransposed on
the host.
"""

import numpy as np
import ml_dtypes

import concourse.tile as tile
import concourse.mybir as mybir
from concourse import bacc
from concourse.bass_utils import run_bass_kernel_spmd

N_CORES = 8
N_TOK = 8192
D_IN = 4096
D_OUT = 4096
P = 128
T = N_TOK // N_CORES  # 1024 tokens per core
KT = D_IN // P  # 32 contraction tiles
MT = D_OUT // P  # 32 out-feature tiles
M2 = 2  # m-tiles per cached W block (256 outf cols -> 1KB DMA lines)
MB = MT // M2  # 16 W blocks
TB = 512  # token block (one PSUM bank of fp32)
NB = T // TB  # 2 token blocks per core

F32 = mybir.dt.float32
BF16 = mybir.dt.bfloat16
SIGN = mybir.ActivationFunctionType.Sign

_nc_cache = None


def build():
    """Build + compile the per-core Bass/Tile module (SPMD: same on all cores)."""
    global _nc_cache
    if _nc_cache is not None:
        return _nc_cache
    nc = bacc.Bacc("TRN2", target_bir_lowering=False, debug=False, num_devices=N_CORES)
    xhi_d = nc.dram_tensor("x_hi_t", [D_IN, T], BF16, kind="ExternalInput").ap()
    xlo_d = nc.dram_tensor("x_lo_t", [D_IN, T], BF16, kind="ExternalInput").ap()
    w_d = nc.dram_tensor("w_t", [D_IN, D_OUT], F32, kind="ExternalInput").ap()
    b_d = nc.dram_tensor("bias", [D_OUT], F32, kind="ExternalInput").ap()
    out_d = nc.dram_tensor("out_t", [D_OUT, T], F32, kind="ExternalOutput").ap()

    with tile.TileContext(nc) as tc:
        with (
            tc.tile_pool(name="x", bufs=1) as x_pool,
            tc.tile_pool(name="wstage", bufs=4) as wstage_pool,
            tc.tile_pool(name="wsb", bufs=2) as w_pool,
            tc.tile_pool(name="bias", bufs=1) as b_pool,
            tc.tile_pool(name="out", bufs=4) as out_pool,
            tc.tile_pool(name="psum", bufs=6, space="PSUM") as psum_pool,
        ):
            # Resident x (both splits), chunked per (split, k-tile, token-block)
            # so matmuls depend on exactly the chunk they read and the PE can
            # start as soon as the first chunks land. gpsimd (SWDGE) keeps
            # these off the sync queue, which streams W.
            xs = [[[None] * NB for _ in range(KT)] for _ in range(2)]
            for ko in range(KT):
                for n in range(NB):
                    for s, src in ((0, xhi_d), (1, xlo_d)):
                        t = x_pool.tile(
                            [P, TB], BF16, tag=f"x{s}_{ko}_{n}", name=f"x{s}_{ko}_{n}"
                        )
                        nc.gpsimd.dma_start(
                            t[:], src[ko * P : (ko + 1) * P, n * TB : (n + 1) * TB]
                        )
                        xs[s][ko][n] = t
            # bias, outf-partition-major: bias_sb[p, mo] = bias[mo*128 + p]
            bias_sb = b_pool.tile([P, MT], F32, tag="bias")
            nc.sync.dma_start(bias_sb[:], b_d.rearrange("(mo p) -> p mo", p=P))

            for mb in range(MB):
                # Stage + sign-convert a [D_IN, 256] W column block to bf16.
                wsb = w_pool.tile([P, KT, M2 * P], BF16, tag="wsb")
                for k in range(KT):
                    wstage = wstage_pool.tile([P, M2 * P], F32, tag="wstage")
                    nc.sync.dma_start(
                        wstage[:],
                        w_d[k * P : (k + 1) * P, mb * M2 * P : (mb + 1) * M2 * P],
                    )
                    nc.scalar.activation(wsb[:, k, :], wstage[:], SIGN)
                for n in range(NB):
                    psums = [
                        psum_pool.tile([P, TB], F32, tag="psum", name=f"psum{mi}")
                        for mi in range(M2)
                    ]
                    for k in range(KT):
                        for mi in range(M2):
                            lhsT = wsb[:, k, mi * P : (mi + 1) * P]
                            nc.tensor.matmul(
                                psums[mi][:],
                                lhsT,
                                xs[0][k][n][:],
                                start=(k == 0),
                                stop=False,
                            )
                            nc.tensor.matmul(
                                psums[mi][:],
                                lhsT,
                                xs[1][k][n][:],
                                start=False,
                                stop=(k == KT - 1),
                            )
                    for mi in range(M2):
                        m = mb * M2 + mi
                        osb = out_pool.tile([P, TB], F32, tag="osb")
                        nc.scalar.activation(
                            osb[:], psums[mi][:], SIGN, bias=bias_sb[:, m : m + 1]
                        )
                        nc.sync.dma_start(
                            out_d[m * P : (m + 1) * P, n * TB : (n + 1) * TB], osb[:]
                        )
    nc.compile()
    _nc_cache = nc
    return nc


def prep_in_maps(x, weight, bias):
    """Host-side layout prep: bf16 hi/lo split of x, transposes, token shards."""
    x = np.asarray(x, dtype=np.float32)
    weight = np.asarray(weight, dtype=np.float32)
    bias = np.asarray(bias, dtype=np.float32)

    x_hi = x.astype(ml_dtypes.bfloat16)
    x_lo = (x - x_hi.astype(np.float32)).astype(ml_dtypes.bfloat16)
    xhi_t = np.ascontiguousarray(x_hi.T)  # [D_IN, N_TOK]
    xlo_t = np.ascontiguousarray(x_lo.T)
    w_t = np.ascontiguousarray(weight.T)  # [D_IN, D_OUT]

    in_maps = []
    for c in range(N_CORES):
        sl = slice(c * T, (c + 1) * T)
        in_maps.append(
            {
                "x_hi_t": np.ascontiguousarray(xhi_t[:, sl]),
                "x_lo_t": np.ascontiguousarray(xlo_t[:, sl]),
                "w_t": w_t,
                "bias": bias,
            }
        )
    return in_maps


def run(x, weight, bias, **spmd_kwargs):
    """Run on the 8 cores; returns (full_output, BassKernelResults)."""
    nc = build()
    in_maps = prep_in_maps(x, weight, bias)
    res = run_bass_kernel_spmd(nc, in_maps, core_ids=list(range(N_CORES)), **spmd_kwargs)
    out = np.empty((N_TOK, D_OUT), dtype=np.float32)
    for c in range(N_CORES):
        out[c * T : (c + 1) * T, :] = res.results[c]["out_t"].T
    return out, res


def kernel(x, weight, bias):
    out, _ = run(x, weight, bias)
    return out


# revision 5
# speedup vs baseline: 1.2888x; 1.2644x over previous
"""Trainium2 Bass kernel for nn_BinaryLinear: out = sign(x @ sign(W).T + bias).

Strategy
--------
Data-parallel over the 8192-token dim: each of the 8 cores gets 1024 tokens
and the full weight matrix.

On-chip compute (per core) is the NT GEMM z.T = sign(W) @ x.T on the
TensorEngine with the contraction (in_features) on the partition dim:

  psum[outf, tok] = sum_k w_b_T[k, outf] * x_T[k, tok]

Both operands are pre-transposed on the host (pure layout prep) so every DMA
is contiguous-per-partition. Precision/speed: x is split as

  x ~= fp16(x) + 2^-6 * e4m3((x - fp16(x)) * 2^6)        (~15-16 mantissa bits)

The hi half runs as regular fp16 matmuls (1 PE cycle/row). The lo half runs
as fp8e4m3 DoubleRow matmuls (0.5 cycles/row, 256-deep contraction per MM)
with the 2^-6 scale folded into the fp8 weights (+-2^-6 is exact in e4m3),
so BOTH halves accumulate into the same fp32 PSUM group with no epilogue
combine. Combined error lands at the fp32 reference's own accumulation-error
scale. fp32 matmul would be 4 cycles/row; a bf16 hi+lo split is 2 cycles/row;
this scheme is 1.5 cycles/row.

sign(W) is computed on-chip (ScalarE Sign: fp32 -> fp16 +-1, then VectorE
*2^-6 -> e4m3). Epilogue fuses bias-add + sign + PSUM->SBUF in one ScalarE
activation (bias is per-partition in the z.T layout). Output is written as
z.T [out_features, tokens] per core and untransposed on the host.

The first four (W-block, token-block) iterations are ordered
(0,n0),(1,n0),(0,n1),(1,n1) so early matmuls only depend on the first half
of the streamed-in resident x, hiding the initial x load entirely.
"""

import numpy as np

import concourse.tile as tile
import concourse.mybir as mybir
from concourse import bacc
from concourse.bass_utils import run_bass_kernel_spmd

N_CORES = 8
N_TOK = 8192
D_IN = 4096
D_OUT = 4096
P = 128
T = N_TOK // N_CORES  # 1024 tokens per core
KT = D_IN // P  # 32 contraction tiles
KP = KT // 2  # 16 DoubleRow k-pairs
MT = D_OUT // P  # 32 out-feature tiles
M2 = 2  # m-tiles per cached W block (256 outf cols)
MB = MT // M2  # 16 W blocks
TB = 512  # token block (one PSUM bank of fp32)
NB = T // TB  # 2 token blocks per core
LO_SCALE = 2.0 ** 6  # host-side scale on the fp8 residual; inverse on weights

F32 = mybir.dt.float32
FP16 = mybir.dt.float16
FP8 = mybir.dt.float8e4
SIGN = mybir.ActivationFunctionType.Sign
DR = mybir.MatmulPerfMode.DoubleRow
E4M3 = mybir.dt.np(FP8)

_nc_cache = None


def build():
    """Build + compile the per-core Bass/Tile module (SPMD: same on all cores)."""
    global _nc_cache
    if _nc_cache is not None:
        return _nc_cache
    nc = bacc.Bacc("TRN2", target_bir_lowering=False, debug=False, num_devices=N_CORES)
    xhi_d = nc.dram_tensor("x_hi_t", [D_IN, T], FP16, kind="ExternalInput").ap()
    xlo_d = nc.dram_tensor("x_lo8_t", [D_IN, T], FP8, kind="ExternalInput").ap()
    w_d = nc.dram_tensor("w_t", [D_IN, D_OUT], F32, kind="ExternalInput").ap()
    b_d = nc.dram_tensor("bias", [D_OUT], F32, kind="ExternalInput").ap()
    out_d = nc.dram_tensor("out_t", [D_OUT, T], F32, kind="ExternalOutput").ap()

    with tile.TileContext(nc) as tc:
        with (
            tc.tile_pool(name="x", bufs=1) as x_pool,
            tc.tile_pool(name="wstage", bufs=4) as wstage_pool,
            tc.tile_pool(name="wsb", bufs=3) as w_pool,
            tc.tile_pool(name="bias", bufs=1) as b_pool,
            tc.tile_pool(name="out", bufs=4) as out_pool,
            tc.tile_pool(name="psum", bufs=6, space="PSUM") as psum_pool,
        ):
            # Resident x, chunked per (k-tile, token-block) so matmuls depend
            # on exactly the chunk they read. gpsimd (SWDGE) keeps these off
            # the sync queue, which streams W. n=0 chunks load first so the
            # reordered first iterations never wait on the n=1 half.
            xhi = [[None] * NB for _ in range(KT)]
            xlo8 = [[None] * NB for _ in range(KP)]
            for n in range(NB):
                for ko in range(KT):
                    th = x_pool.tile([P, TB], FP16, tag=f"xh_{ko}_{n}",
                                     name=f"xh_{ko}_{n}")
                    nc.gpsimd.dma_start(
                        th[:], xhi_d[ko * P : (ko + 1) * P, n * TB : (n + 1) * TB]
                    )
                    xhi[ko][n] = th
                    t2, j = ko // 2, ko % 2
                    if j == 0:
                        tl = x_pool.tile([P, 2, TB], FP8, tag=f"xl_{t2}_{n}",
                                         name=f"xl_{t2}_{n}")
                        xlo8[t2][n] = tl
                    nc.gpsimd.dma_start(
                        xlo8[t2][n][:, j, :],
                        xlo_d[ko * P : (ko + 1) * P, n * TB : (n + 1) * TB],
                    )
            # bias, outf-partition-major: bias_sb[p, mo] = bias[mo*128 + p]
            bias_sb = b_pool.tile([P, MT], F32, tag="bias")
            nc.sync.dma_start(bias_sb[:], b_d.rearrange("(mo p) -> p mo", p=P))

            order = [(0, 0), (1, 0), (0, 1), (1, 1)]
            order += [(mb, n) for mb in range(2, MB) for n in range(NB)]
            wsb_cache = {}

            for mb, n in order:
                if mb not in wsb_cache:
                    # Stage a [D_IN, 256] W column block; convert to
                    # sign() in fp16 (+-1) and e4m3 (+-2^-6).
                    wsb_hi = w_pool.tile([P, KT, M2 * P], FP16, tag="wsb_hi",
                                         name=f"wsb_hi_{mb}")
                    wsb_lo = w_pool.tile([P, KT, M2 * P], FP8, tag="wsb_lo",
                                         name=f"wsb_lo_{mb}")
                    for k in range(KT):
                        wstage = wstage_pool.tile([P, M2 * P], F32, tag="wstage",
                                                  name=f"wstage_{mb}_{k}")
                        nc.sync.dma_start(
                            wstage[:],
                            w_d[k * P : (k + 1) * P, mb * M2 * P : (mb + 1) * M2 * P],
                        )
                        nc.scalar.activation(wsb_hi[:, k, :], wstage[:], SIGN)
                        nc.vector.tensor_scalar_mul(
                            wsb_lo[:, k, :], wsb_hi[:, k, :], 1.0 / LO_SCALE
                        )
                    wsb_cache[mb] = (wsb_hi, wsb_lo)
                wsb_hi, wsb_lo = wsb_cache[mb]

                psums = [
                    psum_pool.tile([P, TB], F32, tag="psum", name=f"ps_{mb}_{n}_{mi}")
                    for mi in range(M2)
                ]
                for t in range(KP):
                    for mi in range(M2):
                        msl = slice(mi * P, (mi + 1) * P)
                        nc.tensor.matmul(
                            psums[mi][:],
                            wsb_hi[:, 2 * t, msl],
                            xhi[2 * t][n][:],
                            start=(t == 0),
                            stop=False,
                        )
                        nc.tensor.matmul(
                            psums[mi][:],
                            wsb_hi[:, 2 * t + 1, msl],
                            xhi[2 * t + 1][n][:],
                            start=False,
                            stop=False,
                        )
                        nc.tensor.matmul(
                            psums[mi][:],
                            wsb_lo[:, 2 * t : 2 * t + 2, msl],
                            xlo8[t][n][:],
                            start=False,
                            stop=(t == KP - 1),
                            perf_mode=DR,
                        )
                for mi in range(M2):
                    m = mb * M2 + mi
                    osb = out_pool.tile([P, TB], F32, tag="osb", name=f"osb_{mb}_{n}_{mi}")
                    nc.scalar.activation(
                        osb[:], psums[mi][:], SIGN, bias=bias_sb[:, m : m + 1]
                    )
                    nc.sync.dma_start(
                        out_d[m * P : (m + 1) * P, n * TB : (n + 1) * TB], osb[:]
                    )
    nc.compile()
    _nc_cache = nc
    return nc


def prep_in_maps(x, weight, bias):
    """Host-side layout prep: fp16/fp8 split of x, transposes, token shards."""
    x = np.asarray(x, dtype=np.float32)
    weight = np.asarray(weight, dtype=np.float32)
    bias = np.asarray(bias, dtype=np.float32)

    x_hi = x.astype(np.float16)
    x_lo8 = ((x - x_hi.astype(np.float32)) * LO_SCALE).astype(E4M3)
    xhi_t = np.ascontiguousarray(x_hi.T)  # [D_IN, N_TOK]
    xlo_t = np.ascontiguousarray(x_lo8.T)
    w_t = np.ascontiguousarray(weight.T)  # [D_IN, D_OUT]

    in_maps = []
    for c in range(N_CORES):
        sl = slice(c * T, (c + 1) * T)
        in_maps.append(
            {
                "x_hi_t": np.ascontiguousarray(xhi_t[:, sl]),
                "x_lo8_t": np.ascontiguousarray(xlo_t[:, sl]),
                "w_t": w_t,
                "bias": bias,
            }
        )
    return in_maps


def run(x, weight, bias, **spmd_kwargs):
    """Run on the 8 cores; returns (full_output, BassKernelResults)."""
    nc = build()
    in_maps = prep_in_maps(x, weight, bias)
    res = run_bass_kernel_spmd(nc, in_maps, core_ids=list(range(N_CORES)), **spmd_kwargs)
    out = np.empty((N_TOK, D_OUT), dtype=np.float32)
    for c in range(N_CORES):
        out[c * T : (c + 1) * T, :] = res.results[c]["out_t"].T
    return out, res


def kernel(x, weight, bias):
    out, _ = run(x, weight, bias)
    return out


# revision 9
# speedup vs baseline: 1.3260x; 1.0289x over previous
"""Trainium2 Bass kernel for nn_BinaryLinear: out = sign(x @ sign(W).T + bias).

Strategy
--------
Data-parallel over the 8192-token dim: each of the 8 cores gets 1024 tokens
and the full weight matrix.

On-chip compute (per core) is the NT GEMM z.T = sign(W) @ x.T on the
TensorEngine with the contraction (in_features) on the partition dim:

  psum[outf, tok] = sum_k w_b_T[k, outf] * x_T[k, tok]

Both operands are pre-transposed on the host (pure layout prep) so every DMA
is contiguous-per-partition. Precision/speed: x is split as

  x ~= fp16(x) + 2^-6 * e4m3((x - fp16(x)) * 2^6)        (~15-16 mantissa bits)

The hi half runs as regular fp16 matmuls (1 PE cycle/row). The lo half runs
as fp8e4m3 DoubleRow matmuls (0.5 cycles/row, 256-deep contraction per MM)
with the 2^-6 scale folded into the fp8 weights (+-2^-6 is exact in e4m3),
so BOTH halves accumulate into the same fp32 PSUM group with no epilogue
combine. Combined error lands at the fp32 reference's own accumulation-error
scale. fp32 matmul would be 4 cycles/row; a bf16 hi+lo split is 2 cycles/row;
this scheme is 1.5 cycles/row.

sign(W) is computed on-chip (ScalarE Sign: fp32 -> fp16 +-1, then VectorE
*2^-6 -> e4m3). Epilogue fuses bias-add + sign + PSUM->SBUF in one ScalarE
activation (bias is per-partition in the z.T layout). Output is written as
z.T [out_features, tokens] per core and untransposed on the host.

The first four (W-block, token-block) iterations are ordered
(0,n0),(1,n0),(0,n1),(1,n1) so early matmuls only depend on the first half
of the streamed-in resident x, hiding the initial x load entirely.
"""

import numpy as np

import concourse.tile as tile
import concourse.mybir as mybir
from concourse import bacc
from concourse.bass_utils import run_bass_kernel_spmd

N_CORES = 8
N_TOK = 8192
D_IN = 4096
D_OUT = 4096
P = 128
T = N_TOK // N_CORES  # 1024 tokens per core
KT = D_IN // P  # 32 contraction tiles
KP = KT // 2  # 16 DoubleRow k-pairs
MT = D_OUT // P  # 32 out-feature tiles
M2 = 2  # m-tiles per cached W block (256 outf cols)
MB = MT // M2  # 16 W blocks
TB = 512  # token block (one PSUM bank of fp32)
NB = T // TB  # 2 token blocks per core
LO_SCALE = 2.0 ** 6  # host-side scale on the fp8 residual; inverse on weights

F32 = mybir.dt.float32
FP16 = mybir.dt.float16
FP8 = mybir.dt.float8e4
SIGN = mybir.ActivationFunctionType.Sign
DR = mybir.MatmulPerfMode.DoubleRow
E4M3 = mybir.dt.np(FP8)

_nc_cache = None


def build():
    """Build + compile the per-core Bass/Tile module (SPMD: same on all cores)."""
    global _nc_cache
    if _nc_cache is not None:
        return _nc_cache
    nc = bacc.Bacc("TRN2", target_bir_lowering=False, debug=False, num_devices=N_CORES)
    xhi_d = nc.dram_tensor("x_hi_t", [D_IN, T], FP16, kind="ExternalInput").ap()
    xlo_d = nc.dram_tensor("x_lo8_t", [D_IN, T], FP8, kind="ExternalInput").ap()
    w_d = nc.dram_tensor("w_t", [D_IN, D_OUT], F32, kind="ExternalInput").ap()
    b_d = nc.dram_tensor("bias", [D_OUT], F32, kind="ExternalInput").ap()
    out_d = nc.dram_tensor("out_t", [D_OUT, T], F32, kind="ExternalOutput").ap()

    with tile.TileContext(nc) as tc:
        with (
            tc.tile_pool(name="x", bufs=1) as x_pool,
            tc.tile_pool(name="wstage", bufs=4) as wstage_pool,
            tc.tile_pool(name="wsb", bufs=3) as w_pool,
            tc.tile_pool(name="bias", bufs=1) as b_pool,
            tc.tile_pool(name="out", bufs=4) as out_pool,
            tc.tile_pool(name="psum", bufs=6, space="PSUM") as psum_pool,
        ):
            def convert_w_block(mb):
                # Stage a [D_IN, 256] W column block; convert to
                # sign() in fp16 (+-1) and e4m3 (+-2^-6).
                wsb_hi = w_pool.tile([P, KT, M2 * P], FP16, tag="wsb_hi",
                                     name=f"wsb_hi_{mb}")
                wsb_lo = w_pool.tile([P, KT, M2 * P], FP8, tag="wsb_lo",
                                     name=f"wsb_lo_{mb}")
                for k in range(KT):
                    wstage = wstage_pool.tile([P, M2 * P], F32, tag="wstage",
                                              name=f"wstage_{mb}_{k}")
                    nc.sync.dma_start(
                        wstage[:],
                        w_d[k * P : (k + 1) * P, mb * M2 * P : (mb + 1) * M2 * P],
                    )
                    nc.scalar.activation(wsb_hi[:, k, :], wstage[:], SIGN)
                    nc.vector.tensor_scalar_mul(
                        wsb_lo[:, k, :], wsb_hi[:, k, :], 1.0 / LO_SCALE
                    )
                return wsb_hi, wsb_lo

            # mb0's W conversion is emitted first so its ScalarE/VectorE ops
            # run ahead of the x-lo DMA stream on the vector queue.
            wsb_cache = {0: convert_w_block(0)}

            # Resident x, chunked per k-tile (full token width) so matmuls
            # depend on exactly the chunk they read, all on the gpsimd queue
            # (the sync queue streams W). hi chunks first in k order, then lo
            # chunks: within each PSUM group all fp16 MMs run before the
            # DoubleRow MMs, so the lo data is needed one hi-phase later and
            # the single serial DMA stream stays ahead of the PE.
            xhi = []
            xlo8 = []
            for ko in range(KT):
                th = x_pool.tile([P, T], FP16, tag=f"xh_{ko}", name=f"xh_{ko}")
                nc.gpsimd.dma_start(th[:], xhi_d[ko * P : (ko + 1) * P, :])
                xhi.append(th)
            for t2 in range(KP):
                tl = x_pool.tile([P, 2, T], FP8, tag=f"xl_{t2}", name=f"xl_{t2}")
                for j in range(2):
                    ko = 2 * t2 + j
                    nc.gpsimd.dma_start(
                        tl[:, j, :], xlo_d[ko * P : (ko + 1) * P, :]
                    )
                xlo8.append(tl)
            # bias, outf-partition-major: bias_sb[p, mo] = bias[mo*128 + p]
            bias_sb = b_pool.tile([P, MT], F32, tag="bias")
            nc.sync.dma_start(bias_sb[:], b_d.rearrange("(mo p) -> p mo", p=P))

            for mb in range(MB):
                if mb not in wsb_cache:
                    wsb_cache[mb] = convert_w_block(mb)
                wsb_hi, wsb_lo = wsb_cache.pop(mb)

                for n in range(NB):
                    nsl = slice(n * TB, (n + 1) * TB)
                    psums = [
                        psum_pool.tile([P, TB], F32, tag="psum",
                                       name=f"ps_{mb}_{n}_{mi}")
                        for mi in range(M2)
                    ]
                    for k in range(KT):
                        for mi in range(M2):
                            msl = slice(mi * P, (mi + 1) * P)
                            nc.tensor.matmul(
                                psums[mi][:],
                                wsb_hi[:, k, msl],
                                xhi[k][:, nsl],
                                start=(k == 0),
                                stop=False,
                            )
                    for t in range(KP):
                        for mi in range(M2):
                            msl = slice(mi * P, (mi + 1) * P)
                            nc.tensor.matmul(
                                psums[mi][:],
                                wsb_lo[:, 2 * t : 2 * t + 2, msl],
                                xlo8[t][:, :, nsl],
                                start=False,
                                stop=(t == KP - 1),
                                perf_mode=DR,
                            )
                    for mi in range(M2):
                        m = mb * M2 + mi
                        osb = out_pool.tile([P, TB], F32, tag="osb",
                                            name=f"osb_{mb}_{n}_{mi}")
                        nc.scalar.activation(
                            osb[:], psums[mi][:], SIGN, bias=bias_sb[:, m : m + 1]
                        )
                        nc.sync.dma_start(
                            out_d[m * P : (m + 1) * P, nsl], osb[:]
                        )
    nc.compile()
    _nc_cache = nc
    return nc


def prep_in_maps(x, weight, bias):
    """Host-side layout prep: fp16/fp8 split of x, transposes, token shards."""
    x = np.asarray(x, dtype=np.float32)
    weight = np.asarray(weight, dtype=np.float32)
    bias = np.asarray(bias, dtype=np.float32)

    x_hi = x.astype(np.float16)
    x_lo8 = ((x - x_hi.astype(np.float32)) * LO_SCALE).astype(E4M3)
    xhi_t = np.ascontiguousarray(x_hi.T)  # [D_IN, N_TOK]
    xlo_t = np.ascontiguousarray(x_lo8.T)
    w_t = np.ascontiguousarray(weight.T)  # [D_IN, D_OUT]

    in_maps = []
    for c in range(N_CORES):
        sl = slice(c * T, (c + 1) * T)
        in_maps.append(
            {
                "x_hi_t": np.ascontiguousarray(xhi_t[:, sl]),
                "x_lo8_t": np.ascontiguousarray(xlo_t[:, sl]),
                "w_t": w_t,
                "bias": bias,
            }
        )
    return in_maps


def run(x, weight, bias, **spmd_kwargs):
    """Run on the 8 cores; returns (full_output, BassKernelResults)."""
    nc = build()
    in_maps = prep_in_maps(x, weight, bias)
    res = run_bass_kernel_spmd(nc, in_maps, core_ids=list(range(N_CORES)), **spmd_kwargs)
    out = np.empty((N_TOK, D_OUT), dtype=np.float32)
    for c in range(N_CORES):
        out[c * T : (c + 1) * T, :] = res.results[c]["out_t"].T
    return out, res


def kernel(x, weight, bias):
    out, _ = run(x, weight, bias)
    return out
